# revision 20
# baseline (speedup 1.0000x reference)
"""DiscreteHazardLoss Trainium2 kernel.

Math
----
reference:  loss_b = -( sum_{j<t} log(1-h_j+eps) + [e=1] log(h_t+eps)
                        + [e=0] log(1-h_t+eps) ),  h = sigmoid(x),  mean over b.
With  log(1-h+eps) ~= -softplus(x)  (eps=1e-7 shift is ~1e-7 relative on the
mean, far below fp32 noise) and  softplus(-x) = softplus(x) - x:

    loss_b = sum_{j<=t_b} softplus(x_bj) - e_b * x_{b,t_b}

Device computes the heavy first term as ln(1 + exp(x)*[j<=t]) summed over
everything (Exp and Ln live in the same ACT table set; a masked element
contributes ln(1) = 0).  Per-row masks come from one tensor_paged_mask per
tile with per-row boundary t+1.  ACT's fused accum_out yields per-partition
sums; the host adds the tiny [128, NT] partials in float64.

The event term sum_b e_b * x_{b,t_b} is one scalar produced by a trivial
gather of the inputs; computed on host in float64.

Sharding: pure data-parallel over the batch axis, 8 cores, 262144 rows each.
"""

import sys

for _p in ("/opt/trn_rl_repo",):
    if _p not in sys.path:
        sys.path.insert(0, _p)

import numpy as np
from contextlib import ExitStack

import concourse.bass as bass
import concourse.bacc as bacc
import concourse.tile as tile
import concourse.mybir as mybir
from concourse.bass_utils import run_bass_kernel_spmd

B, T = 2097152, 32
NCORES = 8
P = 128                      # SBUF partitions
K = 128                      # rows per partition per tile
ROWS_PC = B // NCORES        # 262144 rows per core
NT = ROWS_PC // (P * K)      # 32 tiles per core

_CACHE = {}


def _build_nc(repeat=1):
    nc = bacc.Bacc(
        "TRN2",
        target_bir_lowering=False,
        debug=False,
        enable_asserts=False,
        num_devices=NCORES,
    )
    x_d = nc.dram_tensor("logits", [ROWS_PC, T], mybir.dt.float32, kind="ExternalInput")
    tb_d = nc.dram_tensor("time_bins", [ROWS_PC], mybir.dt.int32, kind="ExternalInput")
    acc_d = nc.dram_tensor("acc", [P, NT], mybir.dt.float32, kind="ExternalOutput")

    x_t = x_d.ap().rearrange("(n p k) t -> n p (k t)", p=P, k=K)   # [NT,128,K*32]
    tb_t = tb_d.ap().rearrange("(n p k) -> n p k", p=P, k=K)       # [NT,128,K]

    with tile.TileContext(nc) as tc, ExitStack() as ctx:
        pool = ctx.enter_context(tc.tile_pool(name="work", bufs=3))
        singles = ctx.enter_context(tc.tile_pool(name="singles", bufs=1))

        acc_tile = singles.tile([P, NT], mybir.dt.float32)

        # one-time: iota over j (value = j); read broadcast over k via step-0
        iota16 = singles.tile([P, T], mybir.dt.int16)
        nc.gpsimd.iota(iota16, pattern=[[1, T]], channel_multiplier=0)
        iotabf = singles.tile([P, T], mybir.dt.bfloat16)
        nc.vector.tensor_copy(iotabf, iota16)

        # one-time: all time_bins for this core in one DMA, then all bounds.
        # bnd2 stores each boundary TWICE (pairs) so the is_gt broadcast reads
        # real step-1 adjacent pairs -> DVE 2x_1P mode stays eligible.
        tbt = singles.tile([P, NT, K], mybir.dt.int32)
        nc.sync.dma_start(
            out=tbt, in_=tb_d.ap().rearrange("(n p k) -> p n k", p=P, k=K)
        )
        bnd2 = singles.tile([P, NT, K, 2], mybir.dt.bfloat16)
        nc.vector.tensor_scalar_add(
            out=bnd2, in0=tbt.unsqueeze(3).broadcast_to([P, NT, K, 2]), scalar1=1
        )

        for n in range(NT * repeat):
            n = n % NT
            xt = pool.tile([P, K * T], mybir.dt.float32, tag="x")
            nc.sync.dma_start(out=xt, in_=x_t[n])

            # ACT pass 1: E = exp(x), contiguous k-major bf16
            e_km = pool.tile([P, K, T], mybir.dt.bfloat16, tag="e")
            nc.scalar.activation(
                out=e_km.rearrange("p a b -> p (a b)"),
                in_=xt,
                func=mybir.ActivationFunctionType.Exp,
            )

            # keep-mask [j <= t] as bf16 0/1:  (t+1) > iota_j
            # bnd read as [k][j-half: step 0][pair: step 1] -> innermost +-1
            bnd_ap = bass.AP(
                tensor=bnd2.tensor,
                offset=bnd2.offset + n * K * 2,
                ap=[bnd2.ap[0], [2, K], [0, T // 2], [1, 2]],
            )
            cmp = pool.tile([P, K, T], mybir.dt.bfloat16, tag="cmp")
            nc.vector.tensor_tensor(
                out=cmp,
                in0=bnd_ap,
                in1=iotabf.unsqueeze(1).broadcast_to([P, K, T]),
                op=mybir.AluOpType.is_gt,
            )

            # E' = E * mask   (both contiguous bf16 -> 2x mode)
            ep = pool.tile([P, K * T], mybir.dt.bfloat16, tag="ep")
            nc.vector.tensor_tensor(
                out=ep,
                in0=e_km.rearrange("p a b -> p (a b)"),
                in1=cmp.rearrange("p a b -> p (a b)"),
                op=mybir.AluOpType.mult,
            )

            # ACT pass 2: ln(E' + 1) summed -> acc column n
            lnout = pool.tile([P, K * T], mybir.dt.bfloat16, tag="lnout")
            nc.scalar.activation(
                out=lnout,
                in_=ep,
                func=mybir.ActivationFunctionType.Ln,
                bias=1.0,
                accum_out=acc_tile[:, n : n + 1],
            )

        nc.sync.dma_start(out=acc_d.ap(), in_=acc_tile)

    # Exp and Ln share one ACT table set; without this the compiler alternates
    # exp_and_others / natural_log per tile (~2.7us per reload, ~90us total).
    # Keep the full dict (act_func_set_id is an index into act_info.json's
    # list) and strip Exp/Ln from every other set so the shared set is chosen.
    _orig_tables = bacc.get_activation_tables

    def _pinned_tables(arch):
        exp_ln = {
            mybir.ActivationFunctionType.Exp,
            mybir.ActivationFunctionType.Ln,
        }
        return {
            name: (funcs if name == "natural_log_exp_and_others" else funcs - exp_ln)
            for name, funcs in _orig_tables(arch).items()
        }

    bacc.get_activation_tables = _pinned_tables
    try:
        nc.compile()
    finally:
        bacc.get_activation_tables = _orig_tables
    return nc


def _get_nc(repeat=1):
    key = ("nc", repeat)
    if key not in _CACHE:
        _CACHE[key] = _build_nc(repeat)
    return _CACHE[key]


def kernel(logits, time_bins, events):
    logits = np.ascontiguousarray(np.asarray(logits, dtype=np.float32))
    tb_i32 = np.ascontiguousarray(
        np.clip(np.asarray(time_bins), 0, T - 1).astype(np.int32)
    )
    events = np.asarray(events, dtype=np.int32)

    nc = _get_nc()
    in_maps = []
    for c in range(NCORES):
        sl = slice(c * ROWS_PC, (c + 1) * ROWS_PC)
        in_maps.append({"logits": logits[sl], "time_bins": tb_i32[sl]})

    res = run_bass_kernel_spmd(nc, in_maps, core_ids=list(range(NCORES)))

    total = 0.0
    for c in range(NCORES):
        total += res.results[c]["acc"].astype(np.float64).sum()

    # event term (tiny scalar derived from inputs; exact in float64)
    x_t = np.take_along_axis(logits, tb_i32[:, None].astype(np.int64), axis=1)[:, 0]
    total -= float(np.where(events == 1, x_t.astype(np.float64), 0.0).sum())

    return np.float32(total / B)


# revision 24
# speedup vs baseline: 1.0145x; 1.0145x over previous
"""DiscreteHazardLoss Trainium2 kernel.

Math
----
reference:  loss_b = -( sum_{j<t} log(1-h_j+eps) + [e=1] log(h_t+eps)
                        + [e=0] log(1-h_t+eps) ),  h = sigmoid(x),  mean over b.
With  log(1-h+eps) ~= -softplus(x)  (eps=1e-7 shift is ~1e-7 relative on the
mean, far below fp32 noise) and  softplus(-x) = softplus(x) - x:

    loss_b = sum_{j<=t_b} softplus(x_bj) - e_b * x_{b,t_b}

Device computes the heavy first term as ln(1 + exp(x)*[j<=t]) summed over
everything (Exp and Ln pinned to the shared natural_log_exp_and_others ACT
table set -> one table load; a masked element contributes ln(1) = 0).
Per-row masks: one DVE is_gt against a per-row boundary t+1 (stored as
adjacent bf16 pairs so the broadcast read keeps 2x_1P mode) then one DVE
mult; all tiles contiguous k-major (strided ACT writes measured 4-5x slow).
ACT's fused accum_out yields per-partition sums; the host adds the tiny
[128, NT] partials in float64.  Measured ~118us/core vs ~100us HBM roofline
(35.6 MB/core at ~358 GB/s); ACT-bound: 2 passes x (4096+352)c/1.2GHz x 16.

The event term sum_b e_b * x_{b,t_b} is one scalar produced by a trivial
gather of the inputs; computed on host in float64.

Sharding: pure data-parallel over the batch axis, 8 cores, 262144 rows each.
"""

import sys

for _p in ("/opt/trn_rl_repo",):
    if _p not in sys.path:
        sys.path.insert(0, _p)

import numpy as np
from contextlib import ExitStack

import concourse.bass as bass
import concourse.bacc as bacc
import concourse.tile as tile
import concourse.mybir as mybir
from concourse.bass_utils import run_bass_kernel_spmd

B, T = 2097152, 32
NCORES = 8
P = 128                      # SBUF partitions
K = 128                      # rows per partition per tile
ROWS_PC = B // NCORES        # 262144 rows per core
NT = ROWS_PC // (P * K)      # 32 tiles per core

_CACHE = {}


def _build_nc(repeat=1):
    nc = bacc.Bacc(
        "TRN2",
        target_bir_lowering=False,
        debug=False,
        enable_asserts=False,
        num_devices=NCORES,
    )
    x_d = nc.dram_tensor("logits", [ROWS_PC, T], mybir.dt.float32, kind="ExternalInput")
    tb_d = nc.dram_tensor("time_bins", [ROWS_PC], mybir.dt.int32, kind="ExternalInput")
    acc_d = nc.dram_tensor("acc", [P, NT], mybir.dt.float32, kind="ExternalOutput")

    x_t = x_d.ap().rearrange("(n p k) t -> n p (k t)", p=P, k=K)   # [NT,128,K*32]
    tb_t = tb_d.ap().rearrange("(n p k) -> n p k", p=P, k=K)       # [NT,128,K]

    with tile.TileContext(nc) as tc, ExitStack() as ctx:
        pool = ctx.enter_context(tc.tile_pool(name="work", bufs=3))
        singles = ctx.enter_context(tc.tile_pool(name="singles", bufs=1))

        acc_tile = singles.tile([P, NT], mybir.dt.float32)

        # one-time: iota over j (value = j); read broadcast over k via step-0
        iota16 = singles.tile([P, T], mybir.dt.int16)
        nc.gpsimd.iota(iota16, pattern=[[1, T]], channel_multiplier=0)
        iotabf = singles.tile([P, T], mybir.dt.bfloat16)
        nc.vector.tensor_copy(iotabf, iota16)

        # one-time: all time_bins for this core in one DMA, then all bounds.
        # bnd2 stores each boundary TWICE (pairs) so the is_gt broadcast reads
        # real step-1 adjacent pairs -> DVE 2x_1P mode stays eligible.
        tbt = singles.tile([P, NT, K], mybir.dt.int32)
        nc.sync.dma_start(
            out=tbt, in_=tb_d.ap().rearrange("(n p k) -> p n k", p=P, k=K)
        )
        bnd2 = singles.tile([P, NT, K, 2], mybir.dt.bfloat16)
        nc.vector.tensor_scalar_add(
            out=bnd2, in0=tbt.unsqueeze(3).broadcast_to([P, NT, K, 2]), scalar1=1
        )

        for n in range(NT * repeat):
            n = n % NT
            xt = pool.tile([P, K * T], mybir.dt.float32, tag="x")
            nc.sync.dma_start(out=xt, in_=x_t[n])

            # ACT pass 1: E = exp(x), contiguous k-major bf16
            e_km = pool.tile([P, K, T], mybir.dt.bfloat16, tag="e")
            nc.scalar.activation(
                out=e_km.rearrange("p a b -> p (a b)"),
                in_=xt,
                func=mybir.ActivationFunctionType.Exp,
            )

            # keep-mask [j <= t] as bf16 0/1:  (t+1) > iota_j
            # bnd read as [k][j-half: step 0][pair: step 1] -> innermost +-1
            bnd_ap = bass.AP(
                tensor=bnd2.tensor,
                offset=bnd2.offset + n * K * 2,
                ap=[bnd2.ap[0], [2, K], [0, T // 2], [1, 2]],
            )
            cmp = pool.tile([P, K, T], mybir.dt.bfloat16, tag="cmp")
            nc.vector.tensor_tensor(
                out=cmp,
                in0=bnd_ap,
                in1=iotabf.unsqueeze(1).broadcast_to([P, K, T]),
                op=mybir.AluOpType.is_gt,
            )

            # E' = E * mask   (both contiguous bf16 -> 2x mode)
            ep = pool.tile([P, K * T], mybir.dt.bfloat16, tag="ep")
            nc.vector.tensor_tensor(
                out=ep,
                in0=e_km.rearrange("p a b -> p (a b)"),
                in1=cmp.rearrange("p a b -> p (a b)"),
                op=mybir.AluOpType.mult,
            )

            # ACT pass 2: ln(E' + 1) summed -> acc column n
            lnout = pool.tile([P, K * T], mybir.dt.bfloat16, tag="lnout")
            nc.scalar.activation(
                out=lnout,
                in_=ep,
                func=mybir.ActivationFunctionType.Ln,
                bias=1.0,
                accum_out=acc_tile[:, n : n + 1],
            )

        nc.sync.dma_start(out=acc_d.ap(), in_=acc_tile)

    # Exp and Ln share one ACT table set; without this the compiler alternates
    # exp_and_others / natural_log per tile (~2.7us per reload, ~90us total).
    # Keep the full dict (act_func_set_id is an index into act_info.json's
    # list) and strip Exp/Ln from every other set so the shared set is chosen.
    _orig_tables = bacc.get_activation_tables

    def _pinned_tables(arch):
        exp_ln = {
            mybir.ActivationFunctionType.Exp,
            mybir.ActivationFunctionType.Ln,
        }
        return {
            name: (funcs if name == "natural_log_exp_and_others" else funcs - exp_ln)
            for name, funcs in _orig_tables(arch).items()
        }

    bacc.get_activation_tables = _pinned_tables
    try:
        nc.compile()
    finally:
        bacc.get_activation_tables = _orig_tables
    return nc


def _get_nc(repeat=1):
    key = ("nc", repeat)
    if key not in _CACHE:
        _CACHE[key] = _build_nc(repeat)
    return _CACHE[key]


def kernel(logits, time_bins, events):
    logits = np.ascontiguousarray(np.asarray(logits, dtype=np.float32))
    tb_i32 = np.ascontiguousarray(
        np.clip(np.asarray(time_bins), 0, T - 1).astype(np.int32)
    )
    events = np.asarray(events, dtype=np.int32)

    nc = _get_nc()
    in_maps = []
    for c in range(NCORES):
        sl = slice(c * ROWS_PC, (c + 1) * ROWS_PC)
        in_maps.append({"logits": logits[sl], "time_bins": tb_i32[sl]})

    res = run_bass_kernel_spmd(nc, in_maps, core_ids=list(range(NCORES)))

    total = 0.0
    for c in range(NCORES):
        total += res.results[c]["acc"].astype(np.float64).sum()

    # event term (tiny scalar derived from inputs; exact in float64)
    x_t = np.take_along_axis(logits, tb_i32[:, None].astype(np.int64), axis=1)[:, 0]
    total -= float(np.where(events == 1, x_t.astype(np.float64), 0.0).sum())

    return np.float32(total / B)
